# revision 21
# baseline (speedup 1.0000x reference)
"""Causal multi-head attention (B=2, H=16, S=2048, D=64, fp32) on 8 TRN2
NeuronCores.

Sharding: the 32 (B*H) head-slices are split 4 per core (pure head
parallelism, no collectives). Each core runs flash-attention-style causal
attention for its 4 heads.

Per-core kernel design (v4):
  * Scores are computed TRANSPOSED: S^T[k, q] = K Q^T, k on partitions, q on
    the free axis, in [128 k, 512 q] tiles, so P^T = exp(S^T) feeds the PV
    matmul directly (contraction = k on partitions) with no O(S^2)
    transposes.
  * ALL matmuls are zero-padded to full-array 128x128xN shape: partial-array
    matmuls (K=64 or M=65) keep the PE activity monitor below its threshold
    and the clock stays at 1.2 GHz; full-array matmuls warm it to 2.4 GHz.
    Zero rows/columns are free (array time is N cycles regardless of K/M).
  * Q/K are transposed on-chip from natural layout with zero columns
    interleaved, so each PE transpose emits the zero-padded [d|0, s] layout
    directly.
  * Softmax runs without max-subtraction; the denominator comes from a ones
    column inside the zero-padded V (row 64 of the PV output).
  * Two 512-query chunks form a superblock that reuses each K/V weight load
    for two matmuls (Ldweights is serialized on this toolchain, ~107 ns
    each).
  * Causal masking is multiplicative (0/1 bf16) after exp on diagonal
    k-tiles only; the row-sum picks it up automatically via the PV matmul.
  * Output drain avoids the PE: DVE normalizes in transposed form
    (partition-broadcast reciprocal row), the xbar DMA transposes bf16
    [64,128] -> [128,64], and a cast-DMA writes fp32 to HBM.
"""

import numpy as np

import concourse.bass as bass
import concourse.mybir as mybir
import concourse.tile as tile
from concourse import bacc
from concourse.bass_utils import run_bass_kernel_spmd

B, H, S, D = 2, 16, 2048, 64
N_CORES = 8
HPC = (B * H) // N_CORES  # heads per core

F32 = mybir.dt.float32
BF16 = mybir.dt.bfloat16
EXP = mybir.ActivationFunctionType.Exp


def _make_identity(nc, ap):
    nc.gpsimd.memset(ap, 0.0)
    sq = ap.shape[0]
    nc.gpsimd.affine_select(
        out=ap,
        in_=ap,
        compare_op=mybir.AluOpType.not_equal,
        fill=1.0,
        base=0,
        pattern=[[-1, sq]],
        channel_multiplier=1,
    )


def build(is_causal: bool, scale: float, s: int = S, hpc: int = HPC):
    QC = 512           # queries per chunk (1 PSUM bank at fp32)
    KT = 128           # keys per tile
    n_qt = s // 128    # s-tiles per head
    n_qc = s // QC     # q-chunks per head

    nc = bacc.Bacc("TRN2", target_bir_lowering=False)
    Qd = nc.declare_dram_parameter("Q", [hpc, s, D], F32, isOutput=False)
    Kd = nc.declare_dram_parameter("K", [hpc, s, D], F32, isOutput=False)
    Vd = nc.declare_dram_parameter("V", [hpc, s, D], F32, isOutput=False)
    Od = nc.declare_dram_parameter("out", [hpc, s, D], F32, isOutput=True)

    with tile.TileContext(nc) as tc:
        _build_body(nc, tc, Qd, Kd, Vd, Od, is_causal, scale, s, hpc,
                    QC, KT, n_qt, n_qc)
    nc.compile()
    return nc


def _build_body(nc, tc, Qd, Kd, Vd, Od, is_causal, scale, s, hpc,
                QC, KT, n_qt, n_qc):
    from contextlib import ExitStack

    with ExitStack() as ctx:
        singles = ctx.enter_context(tc.tile_pool(name="singles", bufs=1))
        nat = ctx.enter_context(tc.tile_pool(name="nat", bufs=2))
        qkt = ctx.enter_context(tc.tile_pool(name="qkt", bufs=4))
        vpool = ctx.enter_context(tc.tile_pool(name="vpool", bufs=2))
        ppool = ctx.enter_context(tc.tile_pool(name="ppool", bufs=3))
        rpool = ctx.enter_context(tc.tile_pool(name="rpool", bufs=4))
        npool = ctx.enter_context(tc.tile_pool(name="npool", bufs=4))
        outpool = ctx.enter_context(tc.tile_pool(name="outpool", bufs=8))
        spool = ctx.enter_context(tc.tile_pool(name="spool", bufs=2, space="PSUM"))
        pvpool = ctx.enter_context(tc.tile_pool(name="pvpool", bufs=4, space="PSUM"))

        ident_b = singles.tile([128, 128], BF16)
        _make_identity(nc, ident_b)

        if is_causal:
            # mask_wide[k, u] = 1.0 iff u - 384 - k >= 0; for a diagonal
            # k-tile with offset o (valid iff q >= k + o, o in {0,128,256,
            # 384}) use slice mask_wide[:, 384-o : 896-o].
            mask = singles.tile([128, 384 + QC], BF16)
            nc.gpsimd.memset(mask, 1.0)
            nc.gpsimd.affine_select(
                out=mask,
                in_=mask,
                compare_op=mybir.AluOpType.is_ge,
                fill=0.0,
                base=-384,
                pattern=[[1, 384 + QC]],
                channel_multiplier=-1,
            )

        # Drains are emitted one superblock late, interleaved into the next
        # superblock's k-loop: their inputs are long since ready, so the
        # in-order DVE/SP queues never stall the PE behind drain latency.
        pending_drains = []

        def emit_drain(pv, h, qc0):
            ov = npool.tile([80, 512], BF16, tag="ov")
            nc.vector.tensor_copy(ov, pv[0:80, :])
            for j in range(4):
                tt = outpool.tile([128, 80], BF16, tag="tt")
                nc.sync.dma_start_transpose(
                    tt, ov[:, 128 * j:128 * (j + 1)])
                rec = rpool.tile([128, 1], F32, tag="rec")
                nc.vector.reciprocal(rec, tt[:, 64:65])
                ot = outpool.tile([128, 64], F32, tag="ot")
                nc.vector.tensor_scalar_mul(ot, tt[:, 0:64], rec)
                nc.gpsimd.dma_start(
                    out=Od[h, qc0 + 128 * j:qc0 + 128 * (j + 1), :],
                    in_=ot)

        for h in range(hpc):
            # ---- natural-layout loads, zero columns interleaved so the PE
            #      transposes emit the zero-padded [d|0, s] layout ----
            qn = nat.tile([128, n_qt, 128], BF16, tag="nat")
            kn = nat.tile([128, n_qt, 128], BF16, tag="nat")
            vp = vpool.tile([128, n_qt, 128], BF16, tag="v")
            nc.gpsimd.memset(qn[:, :, 64:128], 0.0)
            nc.gpsimd.memset(kn[:, :, 64:128], 0.0)
            nc.gpsimd.memset(vp[:, :, 64:128], 0.0)
            nc.gpsimd.memset(vp[:, :, 64:65], 1.0)
            nc.gpsimd.dma_start(
                out=qn[:, :, 0:64],
                in_=Qd[h].rearrange("(t p) d -> p t d", p=128))
            nc.gpsimd.dma_start(
                out=kn[:, :, 0:64],
                in_=Kd[h].rearrange("(t p) d -> p t d", p=128))
            nc.gpsimd.dma_start(
                out=vp[:, :, 0:64],
                in_=Vd[h].rearrange("(t p) d -> p t d", p=128))

            # ---- transpose Q/K to [d|0, s] (rows 64-127 zero) ----
            qt = qkt.tile([128, s], BF16, tag="qkt")
            kt = qkt.tile([128, s], BF16, tag="qkt")
            for src, dst in ((qn, qt), (kn, kt)):
                t = 0
                while t < n_qt:
                    n = min(12, n_qt - t)
                    st = spool.tile([128, 2048], BF16, tag="spsum")
                    for i in range(n):
                        nc.tensor.transpose(
                            st[:, 128 * i:128 * (i + 1)],
                            src[:, t + i, :], ident_b)
                    nc.vector.tensor_copy(
                        dst[:, 128 * t:128 * (t + n)], st[:, :128 * n])
                    t += n

            # ---- main loop: superblocks of 2 chunks (1024 queries) ----
            for sb in range(n_qc // 2):
                q0 = 2 * QC * sb
                n_kt = (q0 + 2 * QC) // KT if is_causal else n_qt
                nc0 = (q0 + QC) // KT if is_causal else n_qt  # c0's k-tiles
                pv0 = pvpool.tile([128, 512], F32, tag="pvst")
                pv1 = pvpool.tile([128, 512], F32, tag="pvst")
                for kti in range(n_kt):
                    # interleave one deferred drain per early k-iteration
                    if kti in (1, 3) and pending_drains:
                        emit_drain(*pending_drains.pop(0))
                    c0 = kti < nc0
                    off = 0 if c0 else 512
                    st = spool.tile([128, 1024], F32, tag="spsum")
                    # QK^T: one K weight load, up to two N=512 matmuls
                    if c0:
                        nc.tensor.matmul(
                            st[:, 0:512],
                            lhsT=kt[:, KT * kti:KT * (kti + 1)],
                            rhs=qt[:, q0:q0 + QC],
                            start=True, stop=True)
                    nc.tensor.matmul(
                        st[:, 512:1024],
                        lhsT=kt[:, KT * kti:KT * (kti + 1)],
                        rhs=qt[:, q0 + QC:q0 + 2 * QC],
                        start=True, stop=True)
                    pt = ppool.tile([128, 1024], BF16, tag="pt")
                    nc.scalar.activation(
                        pt[:, off:1024], st[:, off:1024], EXP, scale=scale)
                    if is_causal:
                        for ci in (0, 1):
                            if ci == 0 and not c0:
                                continue
                            o = KT * kti - (q0 + QC * ci)
                            if 0 <= o < QC:
                                sl = pt[:, 512 * ci:512 * ci + QC]
                                nc.vector.tensor_mul(
                                    sl, sl, mask[:, 384 - o:384 - o + QC])
                    # PV: one V weight load, up to two matmuls; ones column
                    # in V row 64 accumulates the softmax denominators
                    if c0:
                        nc.tensor.matmul(
                            pv0[:, :],
                            lhsT=vp[:, kti, :],
                            rhs=pt[:, 0:512],
                            start=(kti == 0), stop=(kti == nc0 - 1))
                    nc.tensor.matmul(
                        pv1[:, :],
                        lhsT=vp[:, kti, :],
                        rhs=pt[:, 512:1024],
                        start=(kti == 0), stop=(kti == n_kt - 1))

                pending_drains.append((pv0, h, q0))
                pending_drains.append((pv1, h, q0 + QC))

        while pending_drains:
            emit_drain(*pending_drains.pop(0))


def shard_inputs(Q, K, V, s=S, hpc=HPC, n_cores=N_CORES):
    QH = np.ascontiguousarray(np.asarray(Q, np.float32).reshape(-1, s, D))
    KH = np.ascontiguousarray(np.asarray(K, np.float32).reshape(-1, s, D))
    VH = np.ascontiguousarray(np.asarray(V, np.float32).reshape(-1, s, D))
    in_maps = []
    for c in range(n_cores):
        sl = slice(c * hpc, (c + 1) * hpc)
        in_maps.append({
            "Q": np.ascontiguousarray(QH[sl]),
            "K": np.ascontiguousarray(KH[sl]),
            "V": np.ascontiguousarray(VH[sl]),
        })
    return in_maps


def kernel(**inputs) -> np.ndarray:
    Q = np.asarray(inputs["Q"], np.float32)
    K = np.asarray(inputs["K"], np.float32)
    V = np.asarray(inputs["V"], np.float32)
    is_causal = bool(int(np.asarray(inputs["is_causal"])))
    scale = float(np.asarray(inputs["softmax_scale"]))

    in_maps = shard_inputs(Q, K, V)
    nc = build(is_causal, scale)
    res = run_bass_kernel_spmd(nc, in_maps, core_ids=list(range(N_CORES)))
    outs = [res.results[c]["out"] for c in range(N_CORES)]
    return np.concatenate(outs, axis=0).reshape(B, H, S, D).astype(np.float32)


# revision 24
# speedup vs baseline: 1.4793x; 1.4793x over previous
"""Causal multi-head attention (B=2, H=16, S=2048, D=64, fp32) on 8 TRN2
NeuronCores.

Sharding: the 32 (B*H) head-slices are split 4 per core (pure head
parallelism, no collectives). Each core runs flash-attention-style causal
attention for its 4 heads.

Per-core kernel design (v4):
  * Scores are computed TRANSPOSED: S^T[k, q] = K Q^T, k on partitions, q on
    the free axis, in [128 k, 512 q] tiles, so P^T = exp(S^T) feeds the PV
    matmul directly (contraction = k on partitions) with no O(S^2)
    transposes.
  * ALL matmuls are zero-padded to full-array 128x128xN shape: partial-array
    matmuls (K=64 or M=65) keep the PE activity monitor below its threshold
    and the clock stays at 1.2 GHz; full-array matmuls warm it to 2.4 GHz.
    Zero rows/columns are free (array time is N cycles regardless of K/M).
  * Q/K are transposed on-chip from natural layout with zero columns
    interleaved, so each PE transpose emits the zero-padded [d|0, s] layout
    directly.
  * Softmax runs without max-subtraction; the denominator comes from a ones
    column inside the zero-padded V (row 64 of the PV output).
  * Two 512-query chunks form a superblock that reuses each K/V weight load
    for two matmuls (Ldweights is serialized on this toolchain, ~107 ns
    each).
  * Causal masking is multiplicative (0/1 bf16) after exp on diagonal
    k-tiles only; the row-sum picks it up automatically via the PV matmul.
  * Output drain avoids the PE: DVE normalizes in transposed form
    (partition-broadcast reciprocal row), the xbar DMA transposes bf16
    [64,128] -> [128,64], and a cast-DMA writes fp32 to HBM.
"""

import numpy as np

import concourse.bass as bass
import concourse.mybir as mybir
import concourse.tile as tile
from concourse import bacc
from concourse.bass_utils import run_bass_kernel_spmd

B, H, S, D = 2, 16, 2048, 64
N_CORES = 8
HPC = (B * H) // N_CORES  # heads per core

F32 = mybir.dt.float32
BF16 = mybir.dt.bfloat16
EXP = mybir.ActivationFunctionType.Exp


def _make_identity(nc, ap):
    nc.gpsimd.memset(ap, 0.0)
    sq = ap.shape[0]
    nc.gpsimd.affine_select(
        out=ap,
        in_=ap,
        compare_op=mybir.AluOpType.not_equal,
        fill=1.0,
        base=0,
        pattern=[[-1, sq]],
        channel_multiplier=1,
    )


def build(is_causal: bool, scale: float, s: int = S, hpc: int = HPC):
    QC = 512           # queries per chunk (1 PSUM bank at fp32)
    KT = 128           # keys per tile
    n_qt = s // 128    # s-tiles per head
    n_qc = s // QC     # q-chunks per head

    nc = bacc.Bacc("TRN2", target_bir_lowering=False)
    Qd = nc.declare_dram_parameter("Q", [hpc, s, D], F32, isOutput=False)
    Kd = nc.declare_dram_parameter("K", [hpc, s, D], F32, isOutput=False)
    Vd = nc.declare_dram_parameter("V", [hpc, s, D], F32, isOutput=False)
    Od = nc.declare_dram_parameter("out", [hpc, s, D], F32, isOutput=True)

    with tile.TileContext(nc) as tc:
        _build_body(nc, tc, Qd, Kd, Vd, Od, is_causal, scale, s, hpc,
                    QC, KT, n_qt, n_qc)
    nc.compile()
    return nc


def _build_body(nc, tc, Qd, Kd, Vd, Od, is_causal, scale, s, hpc,
                QC, KT, n_qt, n_qc):
    from contextlib import ExitStack

    with ExitStack() as ctx:
        singles = ctx.enter_context(tc.tile_pool(name="singles", bufs=1))
        nat = ctx.enter_context(tc.tile_pool(name="nat", bufs=4))
        qkt = ctx.enter_context(tc.tile_pool(name="qkt", bufs=4))
        vpool = ctx.enter_context(tc.tile_pool(name="vpool", bufs=2))
        ppool = ctx.enter_context(tc.tile_pool(name="ppool", bufs=3))
        rpool = ctx.enter_context(tc.tile_pool(name="rpool", bufs=4))
        npool = ctx.enter_context(tc.tile_pool(name="npool", bufs=4))
        outpool = ctx.enter_context(tc.tile_pool(name="outpool", bufs=8))
        spool = ctx.enter_context(tc.tile_pool(name="spool", bufs=2, space="PSUM"))
        pvpool = ctx.enter_context(tc.tile_pool(name="pvpool", bufs=4, space="PSUM"))

        ident_b = singles.tile([128, 128], BF16)
        _make_identity(nc, ident_b)

        if is_causal:
            # mask_wide[k, u] = 1.0 iff u - 384 - k >= 0; for a diagonal
            # k-tile with offset o (valid iff q >= k + o, o in {0,128,256,
            # 384}) use slice mask_wide[:, 384-o : 896-o].
            mask = singles.tile([128, 384 + QC], BF16)
            nc.gpsimd.memset(mask, 1.0)
            nc.gpsimd.affine_select(
                out=mask,
                in_=mask,
                compare_op=mybir.AluOpType.is_ge,
                fill=0.0,
                base=-384,
                pattern=[[1, 384 + QC]],
                channel_multiplier=-1,
            )

        # Drains are emitted one superblock late, interleaved into the next
        # superblock's k-loop: their inputs are long since ready, so the
        # in-order DVE/SP queues never stall the PE behind drain latency.
        pending_drains = []

        def emit_drain(pv, h, qc0):
            ov = npool.tile([80, 512], BF16, tag="ov")
            nc.vector.tensor_copy(ov, pv[0:80, :])
            # one batched xbar transpose: tt[:, j, :] = ov[:, 128j:128j+128].T
            tt = outpool.tile([128, 4, 80], BF16, tag="tt")
            nc.sync.dma_start_transpose(tt, ov)
            for j in range(4):
                rec = rpool.tile([128, 1], F32, tag="rec")
                nc.vector.reciprocal(rec, tt[:, j, 64:65])
                ot = outpool.tile([128, 64], F32, tag="ot")
                nc.vector.tensor_scalar_mul(ot, tt[:, j, 0:64], rec)
                nc.gpsimd.dma_start(
                    out=Od[h, qc0 + 128 * j:qc0 + 128 * (j + 1), :],
                    in_=ot)

        def emit_prep(h):
            # natural-layout loads with zero columns interleaved, so the PE
            # transposes emit the zero-padded [d|0, s] layout directly
            qn = nat.tile([128, n_qt, 128], BF16, tag="nat", name=f"qn{h}")
            kn = nat.tile([128, n_qt, 128], BF16, tag="nat", name=f"kn{h}")
            vp = vpool.tile([128, n_qt, 128], BF16, tag="v", name=f"vp{h}")
            nc.gpsimd.memset(qn[:, :, 64:128], 0.0)
            nc.gpsimd.memset(kn[:, :, 64:128], 0.0)
            nc.gpsimd.memset(vp[:, :, 64:128], 0.0)
            nc.gpsimd.memset(vp[:, :, 64:65], 1.0)
            nc.gpsimd.dma_start(
                out=qn[:, :, 0:64],
                in_=Qd[h].rearrange("(t p) d -> p t d", p=128))
            nc.gpsimd.dma_start(
                out=kn[:, :, 0:64],
                in_=Kd[h].rearrange("(t p) d -> p t d", p=128))
            nc.gpsimd.dma_start(
                out=vp[:, :, 0:64],
                in_=Vd[h].rearrange("(t p) d -> p t d", p=128))
            # transpose Q/K to [d|0, s] (rows 64-127 zero)
            qt = qkt.tile([128, s], BF16, tag="qkt", name=f"qt{h}")
            kt = qkt.tile([128, s], BF16, tag="qkt", name=f"kt{h}")
            for src, dst in ((qn, qt), (kn, kt)):
                t = 0
                while t < n_qt:
                    n = min(12, n_qt - t)
                    st = spool.tile([128, 2048], BF16, tag="spsum")
                    for i in range(n):
                        nc.tensor.transpose(
                            st[:, 128 * i:128 * (i + 1)],
                            src[:, t + i, :], ident_b)
                    nc.vector.tensor_copy(
                        dst[:, 128 * t:128 * (t + n)], st[:, :128 * n])
                    t += n
            return qt, kt, vp

        preps = {0: emit_prep(0)}

        for h in range(hpc):
            qt, kt, vp = preps.pop(h)

            # ---- main loop: superblocks of 2 chunks (1024 queries) ----
            for sb in range(n_qc // 2):
                q0 = 2 * QC * sb
                n_kt = (q0 + 2 * QC) // KT if is_causal else n_qt
                nc0 = (q0 + QC) // KT if is_causal else n_qt  # c0's k-tiles
                pv0 = pvpool.tile([128, 512], F32, tag="pvst")
                pv1 = pvpool.tile([128, 512], F32, tag="pvst")
                for kti in range(n_kt):
                    # interleave one deferred drain per early k-iteration
                    if kti in (1, 3) and pending_drains:
                        emit_drain(*pending_drains.pop(0))
                    # prefetch the next head's prep mid-way through the
                    # last (largest) superblock of this head
                    if (sb == n_qc // 2 - 1 and kti == n_kt // 2
                            and h + 1 < hpc):
                        preps[h + 1] = emit_prep(h + 1)
                    c0 = kti < nc0
                    off = 0 if c0 else 512
                    st = spool.tile([128, 1024], F32, tag="spsum")
                    # QK^T: one K weight load, up to two N=512 matmuls
                    if c0:
                        nc.tensor.matmul(
                            st[:, 0:512],
                            lhsT=kt[:, KT * kti:KT * (kti + 1)],
                            rhs=qt[:, q0:q0 + QC],
                            start=True, stop=True)
                    nc.tensor.matmul(
                        st[:, 512:1024],
                        lhsT=kt[:, KT * kti:KT * (kti + 1)],
                        rhs=qt[:, q0 + QC:q0 + 2 * QC],
                        start=True, stop=True)
                    pt = ppool.tile([128, 1024], BF16, tag="pt")
                    nc.scalar.activation(
                        pt[:, off:1024], st[:, off:1024], EXP, scale=scale)
                    if is_causal:
                        for ci in (0, 1):
                            if ci == 0 and not c0:
                                continue
                            o = KT * kti - (q0 + QC * ci)
                            if 0 <= o < QC:
                                sl = pt[:, 512 * ci:512 * ci + QC]
                                nc.vector.tensor_mul(
                                    sl, sl, mask[:, 384 - o:384 - o + QC])
                    # PV: one V weight load, up to two matmuls; ones column
                    # in V row 64 accumulates the softmax denominators
                    if c0:
                        nc.tensor.matmul(
                            pv0[:, :],
                            lhsT=vp[:, kti, :],
                            rhs=pt[:, 0:512],
                            start=(kti == 0), stop=(kti == nc0 - 1))
                    nc.tensor.matmul(
                        pv1[:, :],
                        lhsT=vp[:, kti, :],
                        rhs=pt[:, 512:1024],
                        start=(kti == 0), stop=(kti == n_kt - 1))

                pending_drains.append((pv0, h, q0))
                pending_drains.append((pv1, h, q0 + QC))

        while pending_drains:
            emit_drain(*pending_drains.pop(0))


def shard_inputs(Q, K, V, s=S, hpc=HPC, n_cores=N_CORES):
    QH = np.ascontiguousarray(np.asarray(Q, np.float32).reshape(-1, s, D))
    KH = np.ascontiguousarray(np.asarray(K, np.float32).reshape(-1, s, D))
    VH = np.ascontiguousarray(np.asarray(V, np.float32).reshape(-1, s, D))
    in_maps = []
    for c in range(n_cores):
        sl = slice(c * hpc, (c + 1) * hpc)
        in_maps.append({
            "Q": np.ascontiguousarray(QH[sl]),
            "K": np.ascontiguousarray(KH[sl]),
            "V": np.ascontiguousarray(VH[sl]),
        })
    return in_maps


def kernel(**inputs) -> np.ndarray:
    Q = np.asarray(inputs["Q"], np.float32)
    K = np.asarray(inputs["K"], np.float32)
    V = np.asarray(inputs["V"], np.float32)
    is_causal = bool(int(np.asarray(inputs["is_causal"])))
    scale = float(np.asarray(inputs["softmax_scale"]))

    in_maps = shard_inputs(Q, K, V)
    nc = build(is_causal, scale)
    res = run_bass_kernel_spmd(nc, in_maps, core_ids=list(range(N_CORES)))
    outs = [res.results[c]["out"] for c in range(N_CORES)]
    return np.concatenate(outs, axis=0).reshape(B, H, S, D).astype(np.float32)
